# revision 1
# baseline (speedup 1.0000x reference)
"""Fused MHA block (qkvg proj + biased masked softmax + sigmoid gating +
out proj + residual + LayerNorm) for one TRN2 chip — fp8 DoubleRow redesign.

Sharding: data parallel over batch. B=8 -> 8 NeuronCores, one batch element
per core, no collectives. Weights replicated.

Key ideas vs the bf16 baseline:
  * All GEMMs in fp8e4 (e4m3) with MatmulPerfMode.DoubleRow: each instruction
    consumes TWO 128-deep contraction tiles at 0.5 PE cycles per output
    column (4x the bf16 column rate).
  * Scores matmul has only one 128-deep contraction (dh) - it uses a
    "fake" DoubleRow: lhsT/rhs slot 1 is constant zero, still 0.5 cyc/col.
  * The learned bias + mask lands in the scores PSUM via a DoubleRow matmul
    against a block-identity constant: sc += U8^T @ (C*I). Masked entries of
    U8 are -240 (fp8 min) -> exp underflows to exactly 0. This removes the
    entire bf16 CT multiply of the baseline from the vector engines and lets
    ACT write softmax numerators straight to fp8.
  * exp(score)*valid is therefore ONE activation per [128, 1024] PSUM span,
    written directly as the fp8 softmax numerator PT.
  * Softmax denominator: ones[128,2,128]^T @ PT DoubleRow matmuls - the wide
    ones lhsT replicates the denominator across all 128 partitions, so the
    reciprocal (DVE) needs no separate broadcast.
  * v is projected directly in natural [k-token, dh] layout (lhsT = x^T
    tile), killing the per-head PE transposes of the baseline.
  * x^T arrives pre-transposed/pre-quantized from the host (input
    formatting, same spirit as the baseline's host-precomputed exp(bias)).
  * Residual add rides the ff matmul: ff_psum starts from I^T @ (16x in
    bf16), so h = 16*(x+ff) materializes in PSUM; LN stats read PSUM, and
    the normalize is one ACT Identity(scale=rstd, bias=-mu*rstd) per chunk.
    With eps' = 256*eps this is exactly LN(x+ff) by scale invariance.

Scale ledger (fp8 ranges; e4m3 max finite = 240):
    Wq,Wk *8 ; Wv,Wg *64 ; x *1        -> q_ps=8q k_ps=8k v_ps=64v g_ps=64g
    exp arg = sc_ps * ES, ES=1/(64*sqrt(128)); bias via U8=(gamma*b+OFF)*SU,
        SU = 64*sqrt(128)/C, C=128, OFF=-3 (exp <= e^~2.5, fits fp8)
    sig path: t=exp(-g_ps/64)=e^-g (ACT); t2=16*t+16 (Pool bf16);
        sigr=1/t2 = sigmoid(g)/16 (DVE bf16)
    denom_ps = sum_k PT (ones=1);  rd = 1/denom_ps
    ATT = (av_ps * sigr) * rd_bcast = 4*attv*sig   (av_ps = 64*denom*attv)
    W_ff *4 -> ff_ps = 16*ff ; xr = 16*x ; hsb = 16*(x+ff) ; eps' = 256*eps
"""

import math

import numpy as np
import ml_dtypes

import concourse.bass as bass
import concourse.mybir as mybir
import concourse.tile as tile
from concourse import bacc
from concourse.bass_utils import run_bass_kernel_spmd
from concourse.masks import make_identity

B, N, D, H, DH = 8, 1024, 1024, 8, 128
KT = D // 128
KTP = KT // 2
LN_EPS = 1e-5

F32 = mybir.dt.float32
F32R = mybir.dt.float32r
BF16 = mybir.dt.bfloat16
FP8 = mybir.dt.float8e4
DR = mybir.MatmulPerfMode.DoubleRow
FP8NP = ml_dtypes.float8_e4m3

SQ = 8.0        # q,k weight prescale
SV = 64.0       # v,g weight prescale
CID = 128.0     # identity-matmul constant
ES = 1.0 / (SQ * SQ * math.sqrt(DH))     # exp() scale on scores psum
SU = 1.0 / (ES * CID)                    # bias prescale into U8
OFF = -3.0      # score offset (softmax-invariant), keeps exp in fp8 range
SA = 16.0       # hsb = SA*(x+ff)
EPS2 = LN_EPS * SA * SA

_cache = {}


def _build(flags):
    general_gamma, use_bff, use_lng, use_lnb = flags
    nc = bacc.Bacc("TRN2", target_bir_lowering=False)

    xt8_d = nc.dram_tensor("xt8", [128, KT, N], FP8, kind="ExternalInput")
    u8_shape = [H, 128, KT, N] if general_gamma else [128, KT, N]
    u8_d = nc.dram_tensor("u8", u8_shape, FP8, kind="ExternalInput")
    i2_d = nc.dram_tensor("i2", [128, 2, 256], FP8, kind="ExternalInput")
    watt_d = nc.dram_tensor("watt", [H, 128, 4, KTP, 2, 128], FP8, kind="ExternalInput")
    wff_d = nc.dram_tensor("wff", [128, H, D], FP8, kind="ExternalInput")
    xr_d = nc.dram_tensor("xr", [N, D], BF16, kind="ExternalInput")
    if use_bff:
        bff_d = nc.dram_tensor("bff", [1, D], F32, kind="ExternalInput")
    if use_lng:
        lng_d = nc.dram_tensor("lng", [1, D], F32, kind="ExternalInput")
    if use_lnb:
        lnb_d = nc.dram_tensor("lnb", [1, D], F32, kind="ExternalInput")
    out_d = nc.dram_tensor("out", [N, D], BF16, kind="ExternalOutput")

    with tile.TileContext(nc) as tc:
        with (
            tc.tile_pool(name="singles", bufs=1) as singles,
            tc.tile_pool(name="sb_w", bufs=3) as sb_w,
            tc.tile_pool(name="sb_u", bufs=2) as sb_u,
            tc.tile_pool(name="sb_v", bufs=3) as sb_v,
            tc.tile_pool(name="sb_sig", bufs=3) as sb_sig,
            tc.tile_pool(name="sb_rd", bufs=3) as sb_rd,
            tc.tile_pool(name="sb_g1", bufs=4) as sb_g1,
            tc.tile_pool(name="sb_xr", bufs=2) as sb_xr,
            tc.tile_pool(name="sb_h", bufs=2) as sb_h,
            tc.tile_pool(name="sb_st", bufs=4) as sb_st,
            tc.tile_pool(name="sb_o", bufs=3) as sb_o,
            tc.tile_pool(name="ps_pr", bufs=2, space="PSUM") as ps_pr,
            tc.tile_pool(name="ps_tl", bufs=2, space="PSUM") as ps_tl,
            tc.tile_pool(name="ps_sc", bufs=2, space="PSUM") as ps_sc,
        ):
            # ---- constants / resident tensors ----
            XT8 = singles.tile([128, KT, N], FP8, tag="XT8")
            wt0 = singles.tile([128, 4, KTP, 2, 128], FP8, tag="wt0")
            nc.sync.dma_start(out=XT8[:, :, 0:512], in_=xt8_d[:, :, 0:512])
            nc.sync.dma_start(out=wt0, in_=watt_d[0])
            I2 = singles.tile([128, 2, 256], FP8, tag="I2")
            nc.sync.dma_start(out=I2, in_=i2_d[:, :, :])
            WFF8 = singles.tile([128, H, D], FP8, tag="WFF8")
            nc.sync.dma_start(out=XT8[:, :, 512:N], in_=xt8_d[:, :, 512:N])
            U8 = None
            if not general_gamma:
                U8 = singles.tile([128, KT, N], FP8, tag="U8")
                nc.sync.dma_start(out=U8[:, :, 0:512], in_=u8_d[:, :, 0:512])
                nc.sync.dma_start(out=U8[:, :, 512:N], in_=u8_d[:, :, 512:N])
            ones2 = singles.tile([128, 2, 128], FP8, tag="ones2")
            nc.gpsimd.memset(ones2, 1.0)
            eps_t = singles.tile([128, 1], F32, tag="eps_t")
            nc.gpsimd.memset(eps_t, EPS2)
            ATT = singles.tile([128, H, N], FP8, tag="ATT")
            XR = singles.tile([128, KT, D], BF16, tag="XR")
            id_b = singles.tile([128, 128], BF16, tag="id_b")
            make_identity(nc, id_b)
            # zero-slot double-width q/k tiles (slot 1 stays 0 forever)
            NPAR = 4
            q8z = [singles.tile([128, 2, N], FP8, tag=f"q8z{p}", name=f"q8z{p}")
                   for p in range(NPAR)]
            k8z = [singles.tile([128, 2, N], FP8, tag=f"k8z{p}", name=f"k8z{p}")
                   for p in range(NPAR)]
            PT2 = [singles.tile([128, KT, N], FP8, tag=f"PT{p}", name=f"PT{p}")
                   for p in range(NPAR)]
            for p in range(NPAR):
                nc.gpsimd.memset(q8z[p][:, 1, :], 0.0)
                nc.gpsimd.memset(k8z[p][:, 1, :], 0.0)
            if use_bff:
                bffb = singles.tile([128, D], F32, tag="bffb")
                nc.sync.dma_start(
                    out=bffb,
                    in_=bass.AP(tensor=bff_d, offset=0, ap=[[0, 128], [1, D]]),
                )
            if use_lng:
                lngb = singles.tile([128, D], F32, tag="lngb")
                nc.sync.dma_start(
                    out=lngb,
                    in_=bass.AP(tensor=lng_d, offset=0, ap=[[0, 128], [1, D]]),
                )
            if use_lnb:
                lnbb = singles.tile([128, D], F32, tag="lnbb")
                nc.sync.dma_start(
                    out=lnbb,
                    in_=bass.AP(tensor=lnb_d, offset=0, ap=[[0, 128], [1, D]]),
                )

            # ---- per-head attention ----
            for h in range(H):
                par = h % 4
                qz, kz, PT = q8z[par], k8z[par], PT2[par]
                if h == 0:
                    wt = wt0
                else:
                    wt = sb_w.tile([128, 4, KTP, 2, 128], FP8, tag="wt", name="wt")
                    nc.sync.dma_start(out=wt, in_=watt_d[h])
                nc.sync.dma_start(
                    out=XR[:, h, :], in_=xr_d[h * 128 : (h + 1) * 128, :]
                )
                if h == 6:
                    nc.sync.dma_start(out=WFF8, in_=wff_d[:, :, :])
                if general_gamma:
                    U8 = sb_u.tile([128, KT, N], FP8, tag="u8h", name="u8h")
                    nc.sync.dma_start(out=U8, in_=u8_d[h])

                # - q, k projections -> fp8 zero-slot tiles -
                for j, dst in ((0, qz), (1, kz)):
                    for c in range(2):
                        pr = ps_pr.tile([128, 512], F32, tag="ps_pr", name=f"pr_{j}{c}")
                        for ktp in range(KTP):
                            nc.tensor.matmul(
                                pr,
                                wt[:, j, ktp, :, :],
                                XT8[:, 2 * ktp : 2 * ktp + 2, c * 512 : (c + 1) * 512],
                                start=(ktp == 0),
                                stop=(ktp == KTP - 1),
                                perf_mode=DR,
                            )
                        nc.vector.tensor_copy(
                            out=dst[:, 0, c * 512 : (c + 1) * 512], in_=pr
                        )

                # - gate projection -> sigmoid path (stays f32 psum -> ACT) -
                sig_t = sb_sig.tile([128, N], BF16, tag="sig_t", name="sig_t")
                for c in range(2):
                    gr = ps_pr.tile([128, 512], F32, tag="ps_pr", name=f"gr{c}")
                    for ktp in range(KTP):
                        nc.tensor.matmul(
                            gr,
                            wt[:, 3, ktp, :, :],
                            XT8[:, 2 * ktp : 2 * ktp + 2, c * 512 : (c + 1) * 512],
                            start=(ktp == 0),
                            stop=(ktp == KTP - 1),
                            perf_mode=DR,
                        )
                    # t = exp(-g) = exp(g_ps * -1/64)
                    nc.scalar.activation(
                        out=sig_t[:, c * 512 : (c + 1) * 512],
                        in_=gr,
                        func=mybir.ActivationFunctionType.Exp,
                        scale=-1.0 / SV,
                    )
                sig_r = sb_sig.tile([128, N], BF16, tag="sig_r", name="sig_r")
                # t2 = 16*t + 16 ; sigr = 1/t2 = sigmoid(g)/16
                nc.gpsimd.tensor_scalar(
                    sig_r, sig_t, 16.0, 16.0,
                    mybir.AluOpType.mult, mybir.AluOpType.add,
                )
                with nc.allow_low_precision(reason="gate in bf16; |err|~0.4% ok"):
                    nc.vector.reciprocal(sig_r, sig_r)

                # - scores + bias into PSUM, exp -> fp8 PT -
                for kt in range(KT):
                    sc = ps_sc.tile([128, N], F32, tag="ps_sc", name=f"sc{kt}")
                    for qb in range(4):
                        o_ap = sc[:, qb * 256 : (qb + 1) * 256]
                        nc.tensor.matmul(
                            o_ap,
                            U8[:, 2 * qb : 2 * qb + 2, kt * 128 : (kt + 1) * 128],
                            I2,
                            start=True,
                            stop=False,
                            perf_mode=DR,
                        )
                        nc.tensor.matmul(
                            o_ap,
                            kz[:, :, kt * 128 : (kt + 1) * 128],
                            qz[:, :, qb * 256 : (qb + 1) * 256],
                            start=False,
                            stop=True,
                            perf_mode=DR,
                        )
                    nc.scalar.activation(
                        out=PT[:, kt, :],
                        in_=sc,
                        func=mybir.ActivationFunctionType.Exp,
                        scale=ES,
                    )

                # - v projection in natural [k-token, dh] layout -
                v8 = sb_v.tile([128, KT, 128], FP8, tag="v8", name="v8")
                for c in range(2):
                    vr = ps_pr.tile([128, 4, 128], F32, tag="ps_pr", name=f"vr{c}")
                    for nb4 in range(4):
                        nb = 4 * c + nb4
                        for ktp in range(KTP):
                            nc.tensor.matmul(
                                vr[:, nb4, :],
                                XT8[:, 2 * ktp : 2 * ktp + 2, nb * 128 : (nb + 1) * 128],
                                wt[:, 2, ktp, :, :],
                                start=(ktp == 0),
                                stop=(ktp == KTP - 1),
                                perf_mode=DR,
                            )
                    nc.vector.tensor_copy(out=v8[:, 4 * c : 4 * c + 4, :], in_=vr)

                # - denominators (pre-broadcast via ones free=128) + recip -
                rbb = sb_rd.tile([128, N], F32, tag="rbb", name="rbb")
                for c in range(2):
                    dn = ps_tl.tile([128, 512], F32, tag="ps_tl", name=f"dn{c}")
                    for ktp in range(KTP):
                        nc.tensor.matmul(
                            dn,
                            ones2,
                            PT[:, 2 * ktp : 2 * ktp + 2, c * 512 : (c + 1) * 512],
                            start=(ktp == 0),
                            stop=(ktp == KTP - 1),
                            perf_mode=DR,
                        )
                    if h == H - 1 and c == 0:
                        # last head: narrow first 128 cols so nt0's ff
                        # dependency binds early (shortens the tail chain)
                        nc.vector.reciprocal(rbb[:, 0:128], dn[:, 0:128])
                        nc.vector.reciprocal(rbb[:, 128:512], dn[:, 128:512])
                    else:
                        nc.vector.reciprocal(
                            rbb[:, c * 512 : (c + 1) * 512], dn
                        )

                # - attention values av = v8^T @ PT, gating, normalize -
                for c in range(2):
                    av = ps_tl.tile([128, 512], F32, tag="ps_tl", name=f"av{c}")
                    for ktp in range(KTP):
                        nc.tensor.matmul(
                            av,
                            v8[:, 2 * ktp : 2 * ktp + 2, :],
                            PT[:, 2 * ktp : 2 * ktp + 2, c * 512 : (c + 1) * 512],
                            start=(ktp == 0),
                            stop=(ktp == KTP - 1),
                            perf_mode=DR,
                        )
                    t1 = sb_g1.tile([128, 512], BF16, tag="t1", name=f"t1{c}")
                    if h == H - 1 and c == 0:
                        nc.vector.tensor_mul(
                            t1[:, 0:128], av[:, 0:128], sig_r[:, 0:128]
                        )
                        nc.gpsimd.tensor_mul(
                            ATT[:, h, 0:128], t1[:, 0:128], rbb[:, 0:128]
                        )
                        nc.vector.tensor_mul(
                            t1[:, 128:512], av[:, 128:512], sig_r[:, 128:512]
                        )
                        nc.gpsimd.tensor_mul(
                            ATT[:, h, 128:512],
                            t1[:, 128:512],
                            rbb[:, 128:512],
                        )
                    else:
                        nc.vector.tensor_mul(
                            t1, av, sig_r[:, c * 512 : (c + 1) * 512]
                        )
                        nc.gpsimd.tensor_mul(
                            ATT[:, h, c * 512 : (c + 1) * 512],
                            t1,
                            rbb[:, c * 512 : (c + 1) * 512],
                        )

            # ---- output projection + residual + LayerNorm ----
            for nt in range(N // 128):
                ffs = []
                stats = sb_st.tile([128, 2, 6], F32, tag="stats", name="stats")
                r3 = nt % 3
                if r3 == 2:
                    ffpair = ps_sc.tile([128, N], F32, tag="ps_sc", name="ffp")
                for c in range(2):
                    if r3 == 0:
                        ff = ps_pr.tile([128, 512], F32, tag="ps_pr", name=f"ff{c}")
                    elif r3 == 1:
                        ff = ps_tl.tile([128, 512], F32, tag="ps_tl", name=f"ff{c}")
                    else:
                        ff = ffpair[:, c * 512 : (c + 1) * 512]
                    ffs.append(ff)
                    # residual: ff += I^T @ xr16 rows (bf16 matmul into the
                    # same accumulation group)
                    nc.tensor.matmul(
                        ff,
                        id_b,
                        XR[:, nt, c * 512 : (c + 1) * 512],
                        start=True,
                        stop=False,
                    )
                    for fp4 in range(KTP):
                        nc.tensor.matmul(
                            ff,
                            ATT[:, 2 * fp4 : 2 * fp4 + 2, nt * 128 : (nt + 1) * 128],
                            WFF8[:, 2 * fp4 : 2 * fp4 + 2, c * 512 : (c + 1) * 512],
                            start=False,
                            stop=(fp4 == KTP - 1),
                            perf_mode=DR,
                        )
                    if use_bff:
                        nc.vector.tensor_add(
                            ff, ff, bffb[:, c * 512 : (c + 1) * 512]
                        )
                    nc.vector.bn_stats(out=stats[:, c, :], in_=ff)
                mv = sb_st.tile([128, 2], F32, tag="mv", name="mv")
                nc.vector.bn_aggr(out=mv, in_=stats)
                rstd = sb_st.tile([128, 1], F32, tag="rstd", name="rstd")
                nc.scalar.activation(
                    out=rstd,
                    in_=mv[:, 1:2],
                    func=mybir.ActivationFunctionType.Abs_reciprocal_sqrt,
                    bias=eps_t,
                    scale=1.0,
                )
                mb = sb_st.tile([128, 1], F32, tag="mb", name="mb")
                nc.vector.tensor_scalar(
                    mb, mv[:, 0:1], -1.0, rstd,
                    mybir.AluOpType.mult, mybir.AluOpType.mult,
                )
                o = sb_o.tile([128, D], BF16, tag="o", name="o")
                for c in range(2):
                    nc.scalar.activation(
                        out=o[:, c * 512 : (c + 1) * 512],
                        in_=ffs[c],
                        func=mybir.ActivationFunctionType.Identity,
                        bias=mb,
                        scale=rstd,
                    )
                    if use_lng:
                        nc.vector.tensor_mul(
                            o[:, c * 512 : (c + 1) * 512],
                            o[:, c * 512 : (c + 1) * 512],
                            lngb[:, c * 512 : (c + 1) * 512],
                        )
                    if use_lnb:
                        nc.vector.tensor_add(
                            o[:, c * 512 : (c + 1) * 512],
                            o[:, c * 512 : (c + 1) * 512],
                            lnbb[:, c * 512 : (c + 1) * 512],
                        )
                    nc.sync.dma_start(
                        out=out_d[
                            nt * 128 : (nt + 1) * 128, c * 512 : (c + 1) * 512
                        ],
                        in_=o[:, c * 512 : (c + 1) * 512],
                    )

    nc.finalize()
    return nc


def get_nc(flags=(False, False, False, False)):
    if flags not in _cache:
        _cache[flags] = _build(flags)
    return _cache[flags]


def _fp8(a):
    return np.asarray(a, dtype=np.float32).astype(FP8NP)


def kernel(x, mask, bias, gamma_f, W_att, W_ff, b_ff, ln_g, ln_b):
    x = np.asarray(x, dtype=np.float32)
    mask = np.asarray(mask)
    bias = np.asarray(bias, dtype=np.float32)
    gamma_f = np.asarray(gamma_f, dtype=np.float32)
    W_att = np.asarray(W_att, dtype=np.float32)
    W_ff = np.asarray(W_ff, dtype=np.float32)
    b_ff = np.asarray(b_ff, dtype=np.float32)
    ln_g = np.asarray(ln_g, dtype=np.float32)
    ln_b = np.asarray(ln_b, dtype=np.float32)

    general_gamma = not np.all(gamma_f == 1.0)
    use_bff = bool(np.any(b_ff != 0.0))
    use_lng = not np.all(ln_g == 1.0)
    use_lnb = bool(np.any(ln_b != 0.0))
    flags = (general_gamma, use_bff, use_lng, use_lnb)
    nc = get_nc(flags)

    # watt8[h, p, j, ktp, i, fcol] = W_att[ktp*256 + i*128 + p, sect_j + h*128
    #   + fcol] * scale_j   (j: 0=q 1=k 2=v 3=g)
    w4 = W_att.reshape(KTP, 2, 128, 4, H, DH)  # [ktp, i, p, sect, h, fcol]
    watt8 = np.empty((H, 128, 4, KTP, 2, 128), dtype=FP8NP)
    scales = (SQ, SQ, SV, SV)
    for j in range(4):
        # -> [h, p, ktp, i, fcol]
        sect = np.transpose(w4[:, :, :, j, :, :], (3, 2, 0, 1, 4))
        watt8[:, :, j, :, :, :] = _fp8(sect * scales[j])

    wff8 = _fp8(4.0 * W_ff.reshape(H, 128, D).transpose(1, 0, 2))

    # identity constant [p, slot, q] = CID where q == slot*128 + p
    i2 = np.zeros((128, 2, 256), dtype=FP8NP)
    idx = np.arange(128)
    i2[idx, 0, idx] = CID
    i2[idx, 1, 128 + idx] = CID

    valid = ~mask[:, 0, :, :]  # [B, N, N] True where kept

    in_maps = []
    for b in range(B):
        # x^T tiled: XT8[p, kt, n] = x[n, kt*128 + p]
        xt8 = _fp8(np.ascontiguousarray(
            x[b].T.reshape(KT, 128, N).transpose(1, 0, 2)
        ))
        # U8[p, jt, k] = (gamma*b[jt*128+p, k] + OFF)*SU, masked -> -240
        bt = bias[b].reshape(KT, 128, N).transpose(1, 0, 2)  # [p, jt, k]
        vt = valid[b].reshape(KT, 128, N).transpose(1, 0, 2)
        if general_gamma:
            u8 = np.empty((H, 128, KT, N), dtype=FP8NP)
            for h in range(H):
                uh = np.clip((gamma_f[h] * bt + OFF) * SU, -239.0, 239.0)
                u8[h] = np.where(vt, uh, np.float32(-240.0)).astype(FP8NP)
        else:
            uh = np.clip((bt + OFF) * SU, -239.0, 239.0)
            u8 = np.where(vt, uh, np.float32(-240.0)).astype(FP8NP)
        im = {
            "xt8": xt8,
            "u8": u8,
            "i2": i2,
            "watt": watt8,
            "wff": wff8,
            "xr": (SA * x[b]).astype(ml_dtypes.bfloat16),
        }
        if use_bff:
            im["bff"] = SA * b_ff.reshape(1, D)
        if use_lng:
            im["lng"] = ln_g.reshape(1, D)
        if use_lnb:
            im["lnb"] = ln_b.reshape(1, D)
        in_maps.append(im)

    res = run_bass_kernel_spmd(nc, in_maps, core_ids=list(range(B)))
    out = np.stack([res.results[b]["out"] for b in range(B)], axis=0)
    return out.astype(np.float32)

